# revision 2
# baseline (speedup 1.0000x reference)
"""Min-Euclidean-distance retrieval kernel for Trainium2 (8 NeuronCores).

Reference computation:
    x: [1, 2048, 512], y: [1, 65536, 512] (fp32)
    sq[p, r] = ||x_p||^2 + ||y_r||^2 - 2 <x_p, y_r>
    out = min over (p, r) of sqrt(max(sq, 0))

Sharding: the candidate pool (R) is split across 8 cores, 8192 candidates
each. The host pre-arranges both GEMM operands partition-major in fp8
(DoubleRow) with the -2 factor folded into x, so the PSUM directly holds
H[r, p] = -2<x_p, y_r>.

The device only computes, per 128-candidate tile, the min of H over each
group of 64 queries (queries are pre-sorted by ||x||^2 on the host so each
group spans a narrow ||x||^2 range). The norm terms never touch the device:
the host turns the per-group minima into lower/upper bounds on
min_{p in g}(||x_p||^2 + ||y_r||^2 + H[r, p]) and exactly recomputes the
few surviving (candidate, group) pairs in float64 (branch-and-bound).

Engine plan per 1024-query half-tile (PE fills PSUM in 4 DoubleRow MMs):
  3 of 4 halves: ScalarE copies PSUM fp32 -> SBUF fp16 (1.2 GHz), then DVE
    min-reduces the fp16 copy at its 2x (2-byte) rate.
  1 of 4 halves: DVE min-reduces straight from PSUM at 1x.
This keeps ScalarE ~107us and DVE ~101us, both under the PE's ~124us
DoubleRow floor, unlike the previous ACT-bias+min-chain epilogue which put
~127us of serial work on ScalarE alone.
"""

import sys

for _p in ("/opt/trn_rl_repo", "/root/.axon_site/_ro/trn_rl_repo"):
    if _p not in sys.path:
        sys.path.append(_p)

import ml_dtypes
import numpy as np

import concourse.bass as bass
import concourse.mybir as mybir
import concourse.tile as tile
from concourse import bacc, bass_utils

P = 2048          # queries
R = 65536         # candidates (full)
D = 512           # feature dim
NCORES = 8
R_LOC = R // NCORES      # 8192 candidates per core
P_CHUNKS = P // 512      # 4 chunks of queries (DMA + matmul granularity)
R_TILES = R_LOC // 128   # 64 stationary tiles of candidates
R_GROUPS = 16            # DMA granularity for y: 512 candidates per group
K_TILES = D // 128       # 4 contraction tiles
HALVES = R_TILES * 2     # 128 half-tiles of [128 cand x 1024 queries]
QGRP = 64                # query group size for the device-side min
NGRP_H = 1024 // QGRP    # 16 groups per half-tile
NGRP = P // QGRP         # 32 groups over all queries

# Bound slack for the host-side branch-and-bound: covers fp8 GEMM noise on
# H (sigma ~1 on a 512-dim dot) plus fp16 rounding of the staged copies.
SLACK = np.float64(12.0)

F32 = mybir.dt.float32
F16 = mybir.dt.float16
MM_DT = mybir.dt.float8e4
MM_NP = ml_dtypes.float8_e4m3


def _build_module() -> bass.Bass:
    nc = bacc.Bacc("TRN2", target_bir_lowering=False, debug=False)

    # Host-prepared layouts (partition-major, contiguous per partition):
    #   xt[q, c, k, j] = -2 * x_sorted[c*512 + j, k*128 + q]
    #   yt[q, g, k, s] = y[g*512 + s, k*128 + q]
    xt = nc.dram_tensor("xt", [128, P_CHUNKS, K_TILES, 512], MM_DT,
                        kind="ExternalInput")
    yt = nc.dram_tensor("yt", [128, R_GROUPS, K_TILES, 512], MM_DT,
                        kind="ExternalInput")
    # res[lane, h, g] = min over the g-th group of 64 (sorted) queries of
    # H[r, p] for candidate r = (h//2)*128 + lane, query half h%2.
    out = nc.dram_tensor("out", [128, HALVES, NGRP_H], F16,
                         kind="ExternalOutput")

    with tile.TileContext(nc) as tc:
        with (
            tc.tile_pool(name="big", bufs=1) as big,
            tc.tile_pool(name="hbuf", bufs=3) as hpool,
            tc.tile_pool(name="warm", bufs=1) as warm,
            tc.tile_pool(name="psum", bufs=4, space="PSUM") as psum,
        ):
            xt_sb = big.tile([128, P_CHUNKS, K_TILES, 512], MM_DT)
            yt_sb = big.tile([128, R_GROUPS, K_TILES, 512], MM_DT)
            res = big.tile([128, HALVES, NGRP_H], F16)

            # x on the scalar HWDGE ring, y per-group on the sync ring so
            # matmuls unblock progressively.
            nc.scalar.dma_start(xt_sb[:, 0], xt.ap()[:, 0])
            nc.sync.dma_start(yt_sb[:, 0], yt.ap()[:, 0])
            for g in range(1, R_GROUPS):
                nc.sync.dma_start(yt_sb[:, g], yt.ap()[:, g])
            for c in range(1, P_CHUNKS):
                nc.scalar.dma_start(xt_sb[:, c], xt.ap()[:, c])

            # Warm the PE's HAM clock gate during the initial DMA wait with
            # throwaway matmuls on a zeroed scratch tile (~2us of activity
            # brings the 4096-cycle activity window up before real work).
            wsrc = warm.tile([128, 2, 512], MM_DT)
            nc.gpsimd.memset(wsrc[:], 0)
            wp = psum.tile([128, 16, 64], F32, name="pt")
            for _ in range(8):
                nc.tensor.matmul(
                    wp[:, 0:8, :],
                    lhsT=wsrc[:, :, 0:128],
                    rhs=wsrc[:],
                    start=True,
                    stop=True,
                    perf_mode=mybir.MatmulPerfMode.DoubleRow,
                )

            for t in range(R_TILES):
                g, o = t // 4, (t % 4) * 128
                for hh in range(2):
                    h = t * 2 + hh
                    pt = psum.tile([128, NGRP_H, QGRP], F32, name="pt")
                    for ci in range(2):
                        c = hh * 2 + ci
                        for kk in range(K_TILES // 2):
                            nc.tensor.matmul(
                                pt[:, ci * 8 : (ci + 1) * 8, :],
                                lhsT=yt_sb[:, g, 2 * kk : 2 * kk + 2, o : o + 128],
                                rhs=xt_sb[:, c, 2 * kk : 2 * kk + 2, :],
                                start=(kk == 0),
                                stop=(kk == K_TILES // 2 - 1),
                                perf_mode=mybir.MatmulPerfMode.DoubleRow,
                            )
                    if h % 4 == 3:
                        # DVE drains this half directly from PSUM (1x fp32).
                        nc.vector.tensor_reduce(
                            out=res[:, h, :],
                            in_=pt[:],
                            axis=mybir.AxisListType.X,
                            op=mybir.AluOpType.min,
                        )
                    else:
                        # ScalarE drains PSUM to fp16; DVE reduces the fp16
                        # copy at its 2-byte 2x rate.
                        hb = hpool.tile([128, NGRP_H, QGRP], F16, name="hb")
                        nc.scalar.activation(
                            out=hb[:],
                            in_=pt[:],
                            func=mybir.ActivationFunctionType.Copy,
                        )
                        nc.vector.tensor_reduce(
                            out=res[:, h, :],
                            in_=hb[:],
                            axis=mybir.AxisListType.X,
                            op=mybir.AluOpType.min,
                        )
                    # Ship results in quarters so the output DMA overlaps
                    # compute instead of trailing the kernel.
                    if h % 32 == 31:
                        q = h // 32
                        nc.sync.dma_start(
                            out.ap()[:, q * 32 : (q + 1) * 32, :],
                            res[:, q * 32 : (q + 1) * 32, :],
                        )
    nc.compile()
    return nc


_module_cache: bass.Bass | None = None


def _get_module() -> bass.Bass:
    global _module_cache
    if _module_cache is None:
        _module_cache = _build_module()
    return _module_cache


def _to_partition_major(at: np.ndarray, nchunks: int) -> np.ndarray:
    """[D, W] transposed operand -> [128, nchunks, K_TILES, 512] fp8."""
    w = at.shape[1]
    a4 = at.reshape(K_TILES, 128, nchunks, w // nchunks)
    return np.ascontiguousarray(a4.transpose(1, 2, 0, 3).astype(MM_NP))


def _prepare_inputs(x: np.ndarray, y: np.ndarray):
    """Host-side sharding/layout prep. Returns per-core input maps.

    x must already be sorted by ||x||^2 (kernel() does the sort)."""
    xt = _to_partition_major((-2.0 * x).T, P_CHUNKS)
    in_maps = []
    for c in range(NCORES):
        yc = y[c * R_LOC : (c + 1) * R_LOC]
        yct = _to_partition_major(yc.T, R_GROUPS)
        in_maps.append({"xt": xt, "yt": yct})
    return in_maps


def _postprocess(xs: np.ndarray, y: np.ndarray, res: np.ndarray) -> np.ndarray:
    """Branch-and-bound on the device's per-group minima of H = -2<x,y>.

    xs: [P, D] queries sorted by ||x||^2; y: [R, D]; res: [NCORES, 128,
    HALVES, NGRP_H] fp16 group minima. Exact up to the survivor search."""
    xs64 = xs.astype(np.float64)
    y64 = y.astype(np.float64)
    x2 = np.einsum("pd,pd->p", xs64, xs64)
    y2 = np.einsum("rd,rd->r", y64, y64)

    x2g = x2.reshape(NGRP, QGRP)
    x2min, x2max = x2g.min(axis=1), x2g.max(axis=1)

    # hmin[r, gq]: device minimum of H over query group gq for candidate r.
    # res lane/half -> candidate: r = core*8192 + (h//2)*128 + lane,
    # query group gq = (h%2)*NGRP_H + g.
    hm = res.astype(np.float64).reshape(NCORES, 128, R_TILES, 2, NGRP_H)
    hmin = hm.transpose(0, 2, 1, 3, 4).reshape(R, NGRP)

    ub = hmin + y2[:, None] + x2max[None, :] + SLACK
    lb = hmin + y2[:, None] + x2min[None, :] - SLACK
    best_ub = ub.min()
    rs, gs = np.nonzero(lb <= best_ub)

    best = np.inf
    for gq in np.unique(gs):
        rr = rs[gs == gq]
        xg = xs64[gq * QGRP : (gq + 1) * QGRP]
        sq = (
            x2[gq * QGRP : (gq + 1) * QGRP][:, None]
            + y2[rr][None, :]
            - 2.0 * (xg @ y64[rr].T)
        )
        best = min(best, sq.min())
    return np.sqrt(np.float32(max(best, 0.0)))


def kernel(
    predicted_transaction_company: np.ndarray,
    future_transaction_companies_inc_current_data: np.ndarray,
) -> np.ndarray:
    x = np.asarray(predicted_transaction_company, dtype=np.float32)[0]
    y = np.asarray(future_transaction_companies_inc_current_data, dtype=np.float32)[0]

    # Sort queries by ||x||^2 so each group of 64 spans a narrow norm range
    # (tight branch-and-bound intervals). The min is order-invariant.
    order = np.argsort(np.einsum("pd,pd->p", x, x), kind="stable")
    xs = np.ascontiguousarray(x[order])

    nc = _get_module()
    in_maps = _prepare_inputs(xs, y)
    res = bass_utils.run_bass_kernel_spmd(nc, in_maps, core_ids=list(range(NCORES)))
    accs = np.stack([r["out"] for r in res.results])
    return _postprocess(xs, y, accs)


# revision 6
# speedup vs baseline: 1.2385x; 1.2385x over previous
"""Min-Euclidean-distance retrieval kernel for Trainium2 (8 NeuronCores).

Reference computation:
    x: [1, 2048, 512], y: [1, 65536, 512] (fp32)
    sq[p, r] = ||x_p||^2 + ||y_r||^2 - 2 <x_p, y_r>
    out = min over (p, r) of sqrt(max(sq, 0))

Sharding: the candidate pool (R) is split across 8 cores, 8192 candidates
each. The host pre-arranges both GEMM operands partition-major in fp8
(DoubleRow) with the -2 factor folded into x, so PSUM directly holds
H[r, p] = -2<x_p, y_r>.

The device reduces H to per-(lane, query-group-of-64) minima, merged over
all candidate tiles. The norm terms never touch the device: queries are
sorted by ||x||^2 and candidates by ||y||^2 (lane-major, so each output
lane covers 64 y2-adjacent candidates), which makes host-side
branch-and-bound intervals tight. The host exactly recomputes the few
surviving (lane, group) cells in float64.

Engine plan (PE: 512 DoubleRow MMs ~124us is the roofline):
  - ScalarE drains most PSUM half-tiles to fp16 SBUF (1.2 GHz copies).
  - DVE drains the rest two-at-a-time via min(PSUM_a, PSUM_b) tensor_tensor
    (its two read ports make the 1x fp32 pass drain two tiles per op).
  - DVE folds every drained fp16 buffer into a per-query-parity accumulator
    with in-place fp16 min (2-byte operands run the DVE at 2x), then one
    segmented min-reduce per parity yields [128 lanes x 16 groups].
This keeps ScalarE ~94us and DVE ~99us, both under the PE floor, unlike
the v1 ACT-bias epilogue (ScalarE 127us serial) or a tensor_reduce-based
drain (DVE 146us: reduce never triggers the 2x mode, measured 1207ns/half).
"""

import sys

for _p in ("/opt/trn_rl_repo", "/root/.axon_site/_ro/trn_rl_repo"):
    if _p not in sys.path:
        sys.path.append(_p)

import ml_dtypes
import numpy as np

import concourse.bass as bass
import concourse.mybir as mybir
import concourse.tile as tile
from concourse import bacc, bass_utils

P = 2048          # queries
R = 65536         # candidates (full)
D = 512           # feature dim
NCORES = 8
R_LOC = R // NCORES      # 8192 candidates per core
P_CHUNKS = P // 512      # 4 chunks of queries (DMA + matmul granularity)
R_TILES = R_LOC // 128   # 64 stationary tiles of candidates
R_GROUPS = 16            # DMA granularity for y: 512 candidates per group
K_TILES = D // 128       # 4 contraction tiles
QGRP = 64                # query group size for the device-side min
NGRP_H = 1024 // QGRP    # 16 groups per query parity (half)
NGRP = P // QGRP         # 32 groups over all queries

# Bound slack for the host-side branch-and-bound: covers fp8 GEMM noise on
# H (sigma ~1 on a 512-dim dot) plus fp16 rounding of the staged copies.
SLACK = np.float64(8.0)

F32 = mybir.dt.float32
F16 = mybir.dt.float16
MM_DT = mybir.dt.float8e4
MM_NP = ml_dtypes.float8_e4m3

# Half-tile drain mode: every 4th half is drained by a fused DVE
# acc=min(PSUM, acc) op, the rest by ScalarE copies (load balance). The
# BIR verifier allows at most one PSUM input per DVE instruction.
def _dve_half(h: int) -> bool:
    return h % 4 == 3


def _build_module() -> bass.Bass:
    nc = bacc.Bacc("TRN2", target_bir_lowering=False, debug=False)

    # Host-prepared layouts (partition-major, contiguous per partition):
    #   xt[q, c, k, j] = -2 * x_sorted[c*512 + j, k*128 + q]
    #   yt[q, g, k, s] = y_dev[g*512 + s, k*128 + q]
    # where y_dev[t*128 + l] = (per-core y2-sorted y)[l*64 + t].
    xt = nc.dram_tensor("xt", [128, P_CHUNKS, K_TILES, 512], MM_DT,
                        kind="ExternalInput")
    yt = nc.dram_tensor("yt", [128, R_GROUPS, K_TILES, 512], MM_DT,
                        kind="ExternalInput")
    # res[lane, parity, g] = min over candidate tiles and the g-th group of
    # 64 sorted queries (parity selects queries [p*1024, (p+1)*1024)) of H.
    out = nc.dram_tensor("out", [128, 2, NGRP_H], F16, kind="ExternalOutput")

    with tile.TileContext(nc) as tc:
        with (
            tc.tile_pool(name="big", bufs=1) as big,
            tc.tile_pool(name="node", bufs=6) as npool,
            tc.tile_pool(name="psum", bufs=4, space="PSUM") as psum,
        ):
            xt_sb = big.tile([128, P_CHUNKS, K_TILES, 512], MM_DT)
            yt_sb = big.tile([128, R_GROUPS, K_TILES, 512], MM_DT)
            acc = [
                big.tile([128, NGRP_H, QGRP], F16, name=f"acc{p}")
                for p in range(2)
            ]
            res = big.tile([128, 2, NGRP_H], F16)

            # x on the scalar HWDGE ring, y per-group on the sync ring so
            # matmuls unblock progressively.
            nc.scalar.dma_start(xt_sb[:, 0], xt.ap()[:, 0])
            nc.sync.dma_start(yt_sb[:, 0], yt.ap()[:, 0])
            for g in range(1, R_GROUPS):
                nc.sync.dma_start(yt_sb[:, g], yt.ap()[:, g])
            for c in range(1, P_CHUNKS):
                nc.scalar.dma_start(xt_sb[:, c], xt.ap()[:, c])

            acc_init = [False, False]

            def mms(t: int, hh: int):
                """Fill one PSUM half-tile [128 cand x 1024 queries]."""
                g, o = t // 4, (t % 4) * 128
                pt = psum.tile([128, NGRP_H, QGRP], F32, name="pt")
                for ci in range(2):
                    c = hh * 2 + ci
                    for kk in range(K_TILES // 2):
                        nc.tensor.matmul(
                            pt[:, ci * 8 : (ci + 1) * 8, :],
                            lhsT=yt_sb[:, g, 2 * kk : 2 * kk + 2, o : o + 128],
                            rhs=xt_sb[:, c, 2 * kk : 2 * kk + 2, :],
                            start=(kk == 0),
                            stop=(kk == K_TILES // 2 - 1),
                            perf_mode=mybir.MatmulPerfMode.DoubleRow,
                        )
                return pt

            for t in range(R_TILES):
                for hh in range(2):
                    h = t * 2 + hh
                    pt = mms(t, hh)
                    if not acc_init[hh]:
                        # First producer of this parity seeds the
                        # accumulator via a ScalarE copy.
                        nc.scalar.activation(
                            out=acc[hh][:], in_=pt[:],
                            func=mybir.ActivationFunctionType.Copy)
                        acc_init[hh] = True
                    elif _dve_half(h):
                        # Fused drain+fold: one 1x DVE pass reads PSUM and
                        # the fp16 accumulator and writes the min in place.
                        nc.vector.tensor_tensor(
                            out=acc[hh][:], in0=pt[:], in1=acc[hh][:],
                            op=mybir.AluOpType.min)
                    else:
                        # ScalarE drains to fp16; DVE folds at its 2x
                        # (2-byte SBUF) rate.
                        node = npool.tile([128, NGRP_H, QGRP], F16, name="nd")
                        nc.scalar.activation(
                            out=node[:], in_=pt[:],
                            func=mybir.ActivationFunctionType.Copy)
                        nc.vector.tensor_tensor(
                            out=acc[hh][:], in0=acc[hh][:], in1=node[:],
                            op=mybir.AluOpType.min)

            for hh in range(2):
                nc.vector.tensor_reduce(
                    out=res[:, hh, :],
                    in_=acc[hh][:],
                    axis=mybir.AxisListType.X,
                    op=mybir.AluOpType.min,
                )
            nc.sync.dma_start(out.ap(), res[:])
    nc.compile()
    return nc


_module_cache: bass.Bass | None = None


def _get_module() -> bass.Bass:
    global _module_cache
    if _module_cache is None:
        _module_cache = _build_module()
    return _module_cache


def _to_partition_major(at: np.ndarray, nchunks: int) -> np.ndarray:
    """[D, W] transposed operand -> [128, nchunks, K_TILES, 512] fp8."""
    w = at.shape[1]
    a4 = at.reshape(K_TILES, 128, nchunks, w // nchunks)
    return np.ascontiguousarray(a4.transpose(1, 2, 0, 3).astype(MM_NP))


# Device slot rc = tile*128 + lane holds per-core-sorted candidate
# lane*64 + tile, so each output lane covers 64 y2-adjacent candidates.
_PERM = (np.arange(R_LOC) % 128) * (R_LOC // 128) + np.arange(R_LOC) // 128


def _prepare_inputs(x: np.ndarray, y: np.ndarray):
    """Host-side sharding/layout prep. Returns (per-core input maps,
    per-core y2-sorted candidate arrays). x must already be sorted by
    ||x||^2 (kernel() does the sort)."""
    xt = _to_partition_major((-2.0 * x).T, P_CHUNKS)
    in_maps, ysorts = [], []
    for c in range(NCORES):
        yc = y[c * R_LOC : (c + 1) * R_LOC]
        y2c = np.einsum("rd,rd->r", yc, yc, dtype=np.float64)
        ys = np.ascontiguousarray(yc[np.argsort(y2c, kind="stable")])
        ysorts.append(ys)
        yct = _to_partition_major(ys[_PERM].T, R_GROUPS)
        in_maps.append({"xt": xt, "yt": yct})
    return in_maps, ysorts


def _postprocess(xs: np.ndarray, ysorts: list, res: np.ndarray) -> np.ndarray:
    """Branch-and-bound on the device minima of H = -2<x,y>.

    xs: [P, D] queries sorted by ||x||^2; ysorts: per-core y2-sorted
    candidates; res: [NCORES, 128, 2, NGRP_H] fp16. Exact (float64) on the
    surviving cells."""
    xs64 = xs.astype(np.float64)
    x2 = np.einsum("pd,pd->p", xs64, xs64)
    x2g = x2.reshape(NGRP, QGRP)
    x2min, x2max = x2g.min(axis=1), x2g.max(axis=1)

    ys64 = [ys.astype(np.float64) for ys in ysorts]
    y2s = np.stack([np.einsum("rd,rd->r", ys, ys) for ys in ys64])
    y2cell = y2s.reshape(NCORES, 128, R_LOC // 128)
    y2cmin, y2cmax = y2cell.min(axis=2), y2cell.max(axis=2)

    hmin = res.astype(np.float64).reshape(NCORES, 128, NGRP)
    lb = hmin + y2cmin[:, :, None] + x2min[None, None, :] - SLACK
    ub = hmin + y2cmax[:, :, None] + x2max[None, None, :] + SLACK
    best_ub = ub.min()
    ks, ls, gs = np.nonzero(lb <= best_ub)

    best = np.inf
    for k, l, g in zip(ks, ls, gs):
        xg = xs64[g * QGRP : (g + 1) * QGRP]
        yc = ys64[k][l * (R_LOC // 128) : (l + 1) * (R_LOC // 128)]
        sq = (
            x2[g * QGRP : (g + 1) * QGRP][:, None]
            + y2cell[k, l][None, :]
            - 2.0 * (xg @ yc.T)
        )
        best = min(best, sq.min())
    return np.sqrt(np.float32(max(best, 0.0)))


def kernel(
    predicted_transaction_company: np.ndarray,
    future_transaction_companies_inc_current_data: np.ndarray,
) -> np.ndarray:
    x = np.asarray(predicted_transaction_company, dtype=np.float32)[0]
    y = np.asarray(future_transaction_companies_inc_current_data, dtype=np.float32)[0]

    # Sort queries by ||x||^2 so each group of 64 spans a narrow norm range
    # (tight branch-and-bound intervals). The min is order-invariant.
    order = np.argsort(np.einsum("pd,pd->p", x, x), kind="stable")
    xs = np.ascontiguousarray(x[order])

    nc = _get_module()
    in_maps, ysorts = _prepare_inputs(xs, y)
    res = bass_utils.run_bass_kernel_spmd(nc, in_maps, core_ids=list(range(NCORES)))
    accs = np.stack([r["out"] for r in res.results])
    return _postprocess(xs, ysorts, accs)
